# revision 1
# baseline (speedup 1.0000x reference)
"""Trainium2 Bass kernel for nn_NestedConv (gnn_message_passing), v3.

Math (per b, i):
    Xm      = X[b,i] * mask[b,i,:,None]                  # (N,D), rows k masked
    h1      = relu(Xm @ W1 + b1)                         # (N,D)
    h       = relu(h1 @ W2 + b2)                         # (N,D)
    out[b,i] = (A[b].T @ h) * mask[b,i,:,None]           # (N,D), rows j masked

Sharding: data-parallel over batch dim B=64 across 8 NeuronCores (8 b's each).

Why this is ~1000x the original: the old kernel's transposing HBM DMAs
(rearrange APs) moved 256B-contiguous chunks -> ~262k descriptors/core at
HBM latency (~540ns each) = 141ms. Here ALL HBM traffic is contiguous or
xbar-transposed: the (i,k,d)->(k,i,d) input permutation and the
(j,i,d)->(i,j,d) output permutation are host-side numpy swapaxes during
shard/unshard, and everything on-device moves in multi-KB runs.

Primary pipeline (_build_v3, used when b1 == 0 and b2 == 0, bf16 in/out):
  - X (host-pretransposed to (b,k,i,d) bf16) is loaded via xbar
    DMA-transpose (nc.sync.dma_start_transpose) straight into the MLP1
    moving-operand layout ((i-parity,d) partitions, chunk, k) - no PE
    transposes, no xT eviction. Input split into 16 DMAs for overlap.
  - MLP1: stationary = block-diag [[W1,0],[0,W1]] (bf16), 512-free matmuls
  - relu1 evict PSUM->SBUF, split ACT/DVE (relu_dve groups on DVE), bf16
  - MLP2: stationary = h1^T pair slices, moving = block-diag W2 -> h lands
    in NATURAL (k, (i,d)) layout; relu2 evict split ACT/DVE
  - k-mask applied ONCE on h in SBUF (GpSimd): valid because m in {0,1}
    row-scaling commutes with a bias-free MLP (relu(m*z) = m*relu(z))
  - message passing: stationary = A[b] (bf16, exact 0/1), moving = h
  - j-mask fused into the PSUM->SBUF output evict (DVE) into per-window
    (j, 16 i, d) tiles, each stored immediately (contiguous 2KB runs)
Engine balance at the optimum: DVE ~90%, DMA ~89%, ACT ~86%, PE ~68%.

Fallback (_build_v2, when b1/b2 nonzero): PE-transpose pipeline with the
mask applied to X before the MLP and biases via ACT bias / b2 broadcast add.
"""

import sys

sys.path.insert(0, "/opt/trn_rl_repo")

import numpy as np

B, N, D = 64, 128, 64
NC = 8
BSH = B // NC  # batches per core
G = 8  # root-node tiles per group
NG = N // G  # groups per batch

_built = {}


def _build(b2_nonzero: bool, cfg: dict, bsh: int = BSH, ng: int = NG):
    if cfg.get("pipe", "v2") == "v3":
        assert not b2_nonzero
        return _build_v3(cfg, bsh=bsh, ng=ng)
    return _build_v2(b2_nonzero, cfg, bsh=bsh, ng=ng)


def _build_v2(b2_nonzero: bool, cfg: dict, bsh: int = BSH, ng: int = NG):
    import concourse.bacc as bacc
    import concourse.mybir as mybir
    from concourse import tile
    from concourse.bass_interp import get_hw_module

    f32 = mybir.dt.float32
    f32r = mybir.dt.float32r
    bf16 = mybir.dt.bfloat16
    Relu = mybir.ActivationFunctionType.Relu

    xc = cfg.get("x", "f32r")
    x_dt = {"f32r": f32r, "f32": f32, "bf16": bf16}[xc]
    w1_dt = x_dt  # stationary dtype must match the moving xT dtype
    w2_dt = bf16 if cfg.get("mm2", "bf16") == "bf16" else f32
    a_dt = f32r if cfg.get("mm3", "f32r") == "f32r" else f32
    o_dt = bf16 if cfg.get("out", "f32") == "bf16" else f32

    nc = bacc.Bacc("TRN2", target_bir_lowering=False, debug=False, num_devices=1)

    # XT is X pre-transposed on host: XT[b,k,i,d] = X[b,i,k,d]
    X_d = nc.dram_tensor("XT", [bsh, N, N, D], x_dt, kind="ExternalInput").ap()
    A_d = nc.dram_tensor("A", [bsh, N, N], a_dt, kind="ExternalInput").ap()
    # MT[b,k,i] = mask[b,i,k] (transposed mask; serves input k-mask and output j-mask)
    MT_d = nc.dram_tensor("MT", [bsh, N, N], f32, kind="ExternalInput").ap()
    W1_d = nc.dram_tensor("W1Q", [128, 128], w1_dt, kind="ExternalInput").ap()
    W2_d = nc.dram_tensor("W2Q", [128, 128], w2_dt, kind="ExternalInput").ap()
    B1_d = nc.dram_tensor("B1D", [128, 1], f32, kind="ExternalInput").ap()
    ID_d = nc.dram_tensor("IDN", [128, 128], x_dt, kind="ExternalInput").ap()
    if b2_nonzero:
        B2_d = nc.dram_tensor("B2BC", [128, G * D], f32, kind="ExternalInput").ap()
    # OT is the transposed output: OT[b,j,i,d] = out[b,i,j,d] (host swaps back)
    O_d = nc.dram_tensor("OUT", [bsh, N, N, D], o_dt, kind="ExternalOutput").ap()

    GF = G * D  # free size of one group: 512
    KD = N * D  # 8192

    with tile.TileContext(nc) as tc:
        with (
            tc.tile_pool(name="const", bufs=1) as cpool,
            tc.tile_pool(name="xb", bufs=2) as xbpool,
            tc.tile_pool(name="bmeta", bufs=2) as bmpool,
            tc.tile_pool(name="xm", bufs=3) as xmpool,
            tc.tile_pool(name="xT", bufs=3) as xTpool,
            tc.tile_pool(name="h1", bufs=3) as h1pool,
            tc.tile_pool(name="ht", bufs=3) as htpool,
            tc.tile_pool(name="on", bufs=2) as onpool,
            tc.tile_pool(name="psT", bufs=2, space="PSUM") as psTpool,
            tc.tile_pool(name="psH1", bufs=2, space="PSUM") as psH1pool,
            tc.tile_pool(name="psH", bufs=2, space="PSUM") as psHpool,
            tc.tile_pool(name="psO", bufs=2, space="PSUM") as psOpool,
        ):
            w1q = cpool.tile([128, 128], w1_dt)
            nc.sync.dma_start(w1q[:, :], W1_d)
            w2q = cpool.tile([128, 128], w2_dt)
            nc.sync.dma_start(w2q[:, :], W2_d)
            b1d = cpool.tile([128, 1], f32)
            nc.sync.dma_start(b1d[:, :], B1_d)
            idn = cpool.tile([128, 128], x_dt)
            nc.sync.dma_start(idn[:, :], ID_d)
            if b2_nonzero:
                b2bc = cpool.tile([128, GF], f32)
                nc.sync.dma_start(b2bc[:, :], B2_d)

            for b in range(bsh):
                # whole-batch X load: (k partitions, (i,d) free), 4 MiB contiguous
                xb = xbpool.tile([128, KD], x_dt)
                nc.sync.dma_start(
                    xb[:, :].rearrange("k (i d) -> k i d", i=N),
                    X_d[b],
                )
                at = bmpool.tile([128, N], a_dt, tag="at")
                nc.sync.dma_start(at[:, :], A_d[b])
                mt = bmpool.tile([128, N], f32, tag="mt")
                nc.sync.dma_start(mt[:, :], MT_d[b])

                # whole-batch output tile (j partitions, (i,d) free), stored once
                on = onpool.tile([128, KD], o_dt)

                for g in range(ng):
                    i0 = g * G
                    mtg = mt[:, i0 : i0 + G].unsqueeze(2).broadcast_to([128, G, D])

                    # mask X rows (k) for all 8 tiles in one op
                    # (GpSimd: SBUF->SBUF only; it cannot touch PSUM)
                    xm = xmpool.tile([128, GF], x_dt)
                    nc.gpsimd.tensor_mul(
                        xm[:, :].rearrange("k (i d) -> k i d", i=G),
                        xb[:, :].rearrange("k (i d) -> k i d", i=N)[:, i0 : i0 + G, :],
                        mtg,
                    )

                    # transpose pairs: [XmA|XmB] (k, 2*D) -> (d-stacked, k)
                    psT = psTpool.tile([128, GF], x_dt)
                    for p in range(G // 2):
                        nc.tensor.transpose(
                            psT[:, p * 128 : (p + 1) * 128],
                            xm[:, p * 128 : (p + 1) * 128],
                            idn[:, :],
                        )
                    xT = xTpool.tile([128, GF], w1_dt)
                    nc.vector.tensor_copy(xT[:, :], psT[:, :])

                    # MLP layer 1: block-diag W1 computes both pair halves
                    # in one plain matmul
                    psH1 = psH1pool.tile([128, GF], f32)
                    nc.tensor.matmul(
                        psH1[:, :], w1q[:, :], xT[:, :], start=True, stop=True
                    )
                    h1t = h1pool.tile([128, GF], w2_dt)
                    nc.scalar.activation(h1t[:, :], psH1[:, :], Relu, bias=b1d[:, 0:1])

                    # MLP layer 2: stationary = full h1T pair (128 rows),
                    # moving = full block-diag W2 (128 free); the two column
                    # halves land at (i=2p, d) and (i=2p+1, d) -> h natural
                    psH = psHpool.tile([128, GF], f32)
                    for p in range(G // 2):
                        nc.tensor.matmul(
                            psH[:, p * 128 : (p + 1) * 128],
                            h1t[:, p * 128 : (p + 1) * 128],
                            w2q[:, :],
                            start=True,
                            stop=True,
                        )
                    if b2_nonzero:
                        nc.vector.tensor_add(psH[:, :], psH[:, :], b2bc[:, :])
                    ht = htpool.tile([128, GF], a_dt)
                    nc.scalar.activation(ht[:, :], psH[:, :], Relu)

                    # message passing: out[j, (i,d)] = sum_k A[b][k,j] * h[k, (i,d)]
                    psO = psOpool.tile([128, GF], f32)
                    nc.tensor.matmul(
                        psO[:, :], at[:, :], ht[:, :], start=True, stop=True
                    )

                    # mask output rows (j), fused into PSUM->SBUF copy into
                    # the whole-batch output tile
                    nc.vector.tensor_mul(
                        on[:, i0 * D : (i0 + G) * D].rearrange(
                            "j (i d) -> j i d", i=G
                        ),
                        psO[:, :].rearrange("j (i d) -> j i d", i=G),
                        mtg,
                    )

                # one contiguous 4MB (or 2MB bf16) store per batch
                nc.sync.dma_start(
                    O_d[b],
                    on[:, :].rearrange("j (i d) -> j i d", i=N),
                )

    nc.compile()
    nc.m = get_hw_module(nc.m)
    return nc


def _build_v3(cfg: dict, bsh: int = BSH, ng: int = NG):
    """Fast path (b1 == 0 and b2 == 0 only).

    - X is loaded ALREADY TRANSPOSED via one xbar DMA-transpose per batch
      (bf16): xT[(i-par,d), chunk, k] <- XT[b][k, chunk*128 + (i-par)*64 + d].
      No PE transposes, no PSUM->SBUF xT eviction.
    - The k-mask commutes with the (bias-free) MLP: h(m*x) = m*h(x) for
      m in {0,1}, so it is applied once on ht (SBUF->SBUF, GpSimd/DVE).
    - relu evictions are split ACT/DVE to balance engine load.
    """
    import concourse.bacc as bacc
    import concourse.mybir as mybir
    from concourse import tile
    from concourse.bass_interp import get_hw_module

    f32 = mybir.dt.float32
    bf16 = mybir.dt.bfloat16
    Relu = mybir.ActivationFunctionType.Relu

    o_dt = bf16 if cfg.get("out", "bf16") == "bf16" else f32

    nc = bacc.Bacc("TRN2", target_bir_lowering=False, debug=False, num_devices=1)

    # XT is X pre-transposed on host: XT[b,k,i,d] = X[b,i,k,d], bf16
    X_d = nc.dram_tensor("XT", [bsh, N, N, D], bf16, kind="ExternalInput").ap()
    A_d = nc.dram_tensor("A", [bsh, N, N], bf16, kind="ExternalInput").ap()
    MT_d = nc.dram_tensor("MT", [bsh, N, N], bf16, kind="ExternalInput").ap()
    W1_d = nc.dram_tensor("W1Q", [128, 128], bf16, kind="ExternalInput").ap()
    W2_d = nc.dram_tensor("W2Q", [128, 128], bf16, kind="ExternalInput").ap()
    O_d = nc.dram_tensor("OUT", [bsh, N, N, D], o_dt, kind="ExternalOutput").ap()

    GF = G * D  # 512
    KD = N * D  # 8192
    NCH = KD // 128  # 64 chunks of the transposed X per batch

    # engine-split knobs (out of 16 groups per batch)
    relu_dve = int(cfg.get("relu_dve", 4))  # groups whose relu evict goes to DVE
    mask_dve = int(cfg.get("mask_dve", 3))  # groups whose ht-mask goes to DVE
    og = int(cfg.get("og", 2))  # groups per psO tile (1 or 2)
    aux_sync = cfg.get("aux", "scalar") == "sync"  # aux DMA queue engine
    h1w = int(cfg.get("h1w", 1))  # groups per psH1 tile
    h2w = int(cfg.get("h2w", 1))  # groups per psH tile
    h1b = int(cfg.get("h1b", 2))  # psH1 pool bufs
    h2b = int(cfg.get("h2b", 2))  # psH pool bufs
    ob = int(cfg.get("ob", 2))  # psO pool bufs
    out_alt = int(cfg.get("out_alt", 0))  # og-windows (of wpb) routed ACT+Pool
    wpb = ng // og  # og-windows per batch
    on_win = cfg.get("on_win", "0") == "1"  # per-window output tiles + stores
    meta1 = cfg.get("meta1", "0") == "1"  # load A/MT once for all batches
    relu_mode = cfg.get("relu_mode", "period")  # period | r2even
    stq = cfg.get("stq", "sync")  # store issue queue: sync | gpsimd
    interleave = cfg.get("il", "0") == "1"  # interleave mask with MP emission
    banks = h1w * h1b + h2w * h2b + og * ob
    assert banks <= 8, f"PSUM over budget: {banks}"

    with tile.TileContext(nc) as tc:
        with (
            tc.tile_pool(name="const", bufs=1) as cpool,
            tc.tile_pool(name="xt", bufs=int(cfg.get("xb", 2))) as xtpool,
            tc.tile_pool(name="bmeta", bufs=2) as bmpool,
            tc.tile_pool(name="h1", bufs=int(cfg.get("h1p", 3))) as h1pool,
            tc.tile_pool(name="ht", bufs=int(cfg.get("htb", 2))) as htpool,
            tc.tile_pool(name="on", bufs=int(cfg.get("onb", 2))) as onpool,
            tc.tile_pool(name="psH1", bufs=h1b, space="PSUM") as psH1pool,
            tc.tile_pool(name="psH", bufs=h2b, space="PSUM") as psHpool,
            tc.tile_pool(name="psO", bufs=ob, space="PSUM") as psOpool,
        ):
            w1q = cpool.tile([128, 128], bf16)
            nc.sync.dma_start(w1q[:, :], W1_d)
            w2q = cpool.tile([128, 128], bf16)
            nc.sync.dma_start(w2q[:, :], W2_d)

            relu_off = int(cfg.get("relu_off", 0))
            r1_dve = int(cfg.get("r1_dve", relu_dve))  # per-stage DVE counts
            r2_dve = int(cfg.get("r2_dve", relu_dve))

            def relu_on_dve(g, stage=0):
                if relu_mode == "r2even":
                    # relu1 always ACT; relu2 on DVE for even groups
                    return stage == 1 and g % 2 == 0
                # clustered pattern; periods schedule better than even stripes
                n = r2_dve if stage == 1 else r1_dve
                return n > 0 and (g - relu_off) % max(ng // n, 1) == 0

            def mask_on_dve(g):
                return mask_dve > 0 and g % max(ng // mask_dve, 1) == 0

            nsplit = int(cfg.get("nsplit", 1))  # input/store DMA split factor

            for b in range(bsh):
                # xbar DMA-transpose: (k, (i,d)) DRAM -> ((i-par,d), chunk, k);
                # split for finer overlap with compute
                xt = xtpool.tile([128, NCH, 128], bf16)
                xsrc = X_d[b].rearrange("k i d -> k (i d)")
                for s in range(nsplit):
                    c0, c1 = NCH // nsplit * s, NCH // nsplit * (s + 1)
                    nc.sync.dma_start_transpose(
                        xt[:, c0:c1, :], xsrc[:, c0 * 128 : c1 * 128]
                    )
                dma_aux = nc.sync.dma_start if aux_sync else nc.scalar.dma_start
                dma_meta = (
                    nc.scalar.dma_start if cfg.get("amq", "sync") == "scalar" else dma_aux
                )
                at = bmpool.tile([128, N], bf16, tag="at")
                dma_meta(at[:, :], A_d[b])
                mt = bmpool.tile([128, N], bf16, tag="mt")
                dma_meta(mt[:, :], MT_d[b])

                # whole-batch h (k, (i,d)) and output tile (j, (i,d))
                ht = htpool.tile([128, KD], bf16)
                on = None if on_win else onpool.tile([128, KD], o_dt)

                # process groups in windows of W; inside a window all stage
                # tiles are emitted in data-dependency order
                W = max(h1w, h2w, og)
                assert W % h1w == 0 and W % h2w == 0 and W % og == 0
                assert ng % W == 0
                for w in range(ng // W):
                    g0 = w * W

                    # MLP layer 1 (+ relu1) per h1w-subwindow
                    h1tiles = []
                    for s in range(W // h1w):
                        psH1 = psH1pool.tile([128, h1w * GF], f32)
                        for t in range(h1w):
                            g = g0 + s * h1w + t
                            nc.tensor.matmul(
                                psH1[:, t * GF : (t + 1) * GF],
                                w1q[:, :],
                                xt[:, G // 2 * g : G // 2 * (g + 1), :],
                                start=True,
                                stop=True,
                            )
                        h1t = h1pool.tile([128, h1w * GF], bf16)
                        if relu_on_dve(g0 + s * h1w):
                            nc.vector.tensor_scalar_max(h1t[:, :], psH1[:, :], 0.0)
                        else:
                            nc.scalar.activation(h1t[:, :], psH1[:, :], Relu)
                        h1tiles.append(h1t)

                    # MLP layer 2 (+ relu2) per h2w-subwindow:
                    # stationary = h1T pair slice, moving = block-diag W2
                    for s2 in range(W // h2w):
                        psH = psHpool.tile([128, h2w * GF], f32)
                        for t in range(h2w):
                            gi = s2 * h2w + t
                            h1t = h1tiles[gi // h1w]
                            off = (gi % h1w) * (G // 2)
                            for p in range(G // 2):
                                nc.tensor.matmul(
                                    psH[
                                        :,
                                        t * GF + p * 128 : t * GF + (p + 1) * 128,
                                    ],
                                    h1t[:, (off + p) * 128 : (off + p + 1) * 128],
                                    w2q[:, :],
                                    start=True,
                                    stop=True,
                                )
                        ii = (g0 + s2 * h2w) * G
                        hs2 = ht[:, ii * D : (ii + h2w * G) * D]
                        if relu_on_dve(g0 + s2 * h2w, stage=1):
                            nc.vector.tensor_scalar_max(hs2, psH[:, :], 0.0)
                        else:
                            nc.scalar.activation(hs2, psH[:, :], Relu)

                    # k-mask on h (valid: no biases), SBUF->SBUF, in place
                    def emit_mask(g):
                        i0 = g * G
                        mtg = (
                            mt[:, i0 : i0 + G].unsqueeze(2).broadcast_to([128, G, D])
                        )
                        hview = ht[:, i0 * D : (i0 + G) * D].rearrange(
                            "k (i d) -> k i d", i=G
                        )
                        if mask_on_dve(g):
                            nc.vector.tensor_mul(hview, hview, mtg)
                        else:
                            nc.gpsimd.tensor_mul(hview, hview, mtg)

                    if cfg.get("mw", "1") == "2" and W == 2:
                        # one pair-wide mask op (fewer Pool ops + sem hops)
                        i0 = g0 * G
                        mtgw = (
                            mt[:, i0 : i0 + 2 * G]
                            .unsqueeze(2)
                            .broadcast_to([128, 2 * G, D])
                        )
                        hvw = ht[:, i0 * D : (i0 + 2 * G) * D].rearrange(
                            "k (i d) -> k i d", i=2 * G
                        )
                        if mask_on_dve(g0):
                            nc.vector.tensor_mul(hvw, hvw, mtgw)
                        else:
                            nc.gpsimd.tensor_mul(hvw, hvw, mtgw)
                    elif not interleave:
                        for t in range(W):
                            emit_mask(g0 + t)

                    # message passing + masked output evict per og-subwindow
                    for s3 in range(W // og):
                        psO = psOpool.tile([128, og * GF], f32)
                        for t in range(og):
                            g = g0 + s3 * og + t
                            i0 = g * G
                            if interleave:
                                emit_mask(g)
                            nc.tensor.matmul(
                                psO[:, t * GF : (t + 1) * GF],
                                at[:, :],
                                ht[:, i0 * D : (i0 + G) * D],
                                start=True,
                                stop=True,
                            )
                        i00 = (g0 + s3 * og) * G
                        if on_win:
                            ow = onpool.tile([128, og * GF], o_dt, tag="ow")
                            oslc = ow[:, :]
                        else:
                            oslc = on[:, i00 * D : (i00 + og * G) * D]
                        oview = oslc.rearrange("j (i d) -> j i d", i=og * G)
                        mtg2 = (
                            mt[:, i00 : i00 + og * G]
                            .unsqueeze(2)
                            .broadcast_to([128, og * G, D])
                        )
                        gw = (g0 // og) + s3  # global og-window index
                        if out_alt and gw % max(wpb // out_alt, 1) == 0:
                            # route around DVE: plain ACT evict, then Pool
                            # applies the j-mask in place (SBUF only)
                            nc.scalar.copy(oslc, psO[:, :])
                            nc.gpsimd.tensor_mul(oview, oview, mtg2)
                        else:
                            nc.vector.tensor_mul(
                                oview,
                                psO[:, :].rearrange("j (i d) -> j i d", i=og * G),
                                mtg2,
                            )
                        if on_win:
                            dma_store = (
                                nc.gpsimd.dma_start if stq == "gpsimd" else dma_aux
                            )
                            dma_store(
                                O_d[b, :, i00 : i00 + og * G],
                                oview,
                            )

                if not on_win:
                    for s in range(nsplit):
                        r0, r1 = N // nsplit * s, N // nsplit * (s + 1)
                        dma_aux(
                            O_d[b, :, r0:r1],
                            on[:, r0 * D : r1 * D].rearrange(
                                "j (i d) -> j i d", i=r1 - r0
                            ),
                        )

    nc.compile()
    nc.m = get_hw_module(nc.m)
    return nc


def _host_inputs(X, A, mask, W1, b1, W2, b2, cfg):
    import ml_dtypes

    v3 = cfg.get("pipe", "v2") == "v3"
    x_np = (
        ml_dtypes.bfloat16
        if (v3 or cfg.get("x", "f32r") == "bf16")
        else np.float32
    )
    a_np = ml_dtypes.bfloat16 if v3 else np.float32
    mt_np = ml_dtypes.bfloat16 if v3 else np.float32
    w1_np = x_np
    w2_np = ml_dtypes.bfloat16 if cfg.get("mm2", "bf16") == "bf16" else np.float32

    # pre-transpose X on host: XT[b,k,i,d] = X[b,i,k,d]
    XT = np.ascontiguousarray(np.swapaxes(np.asarray(X, dtype=np.float32), 1, 2)).astype(
        x_np
    )
    A = np.ascontiguousarray(np.asarray(A, dtype=np.float32)).astype(a_np)
    MT = np.ascontiguousarray(np.swapaxes(mask, 1, 2)).astype(mt_np)
    W1 = np.asarray(W1, dtype=np.float32)
    W2 = np.asarray(W2, dtype=np.float32)
    b1 = np.asarray(b1, dtype=np.float32)
    b2 = np.asarray(b2, dtype=np.float32)

    w1q = np.zeros((128, 128), dtype=np.float32)  # block-diag [[W1,0],[0,W1]]
    w1q[0:64, 0:64] = W1
    w1q[64:128, 64:128] = W1
    w1q = w1q.astype(w1_np)
    w2q = np.zeros((128, 128), dtype=np.float32)  # block-diag [[W2,0],[0,W2]]
    w2q[0:64, 0:64] = W2
    w2q[64:128, 64:128] = W2
    w2q = w2q.astype(w2_np)
    b1d = np.concatenate([b1, b1], axis=0).reshape(128, 1).astype(np.float32)
    idn = np.eye(128, dtype=x_np)

    if v3:
        shared = {"W1Q": w1q, "W2Q": w2q}
    else:
        shared = {"W1Q": w1q, "W2Q": w2q, "B1D": b1d, "IDN": idn}
    b2_nonzero = bool(np.any(b2 != 0.0))
    if b2_nonzero and not v3:
        shared["B2BC"] = np.tile(b2, (128, G)).astype(np.float32)
    return XT, A, MT, shared, b2_nonzero


def kernel(X, A, mask, W1, b1, W2, b2):
    import ml_dtypes
    from concourse.bass_utils import run_bass_kernel_spmd

    bias_nonzero = bool(np.any(np.asarray(b1) != 0.0)) or bool(
        np.any(np.asarray(b2) != 0.0)
    )
    if bias_nonzero:
        # general path: mask applied to X before the MLP, biases supported
        cfg = dict(pipe="v2", x="bf16", mm1="f32r", mm2="bf16", mm3="f32r", out="bf16")
    else:
        cfg = dict(
            pipe="v3",
            out="bf16",
            mm2="bf16",
            relu_dve=4,
            mask_dve=0,
            og=2,
            aux="sync",
            nsplit=16,
            on_win="1",
        )

    XT, A, MT, shared, b2_nonzero = _host_inputs(X, A, mask, W1, b1, W2, b2, cfg)

    key = (bias_nonzero, tuple(sorted(cfg.items())))
    if key not in _built:
        _built[key] = _build(b2_nonzero, cfg)
    nc = _built[key]

    in_maps = []
    for c in range(NC):
        sl = slice(c * BSH, (c + 1) * BSH)
        in_maps.append({"XT": XT[sl], "A": A[sl], "MT": MT[sl], **shared})

    try:
        res = run_bass_kernel_spmd(nc, in_maps, core_ids=list(range(NC)))
    except Exception:
        res = run_bass_kernel_spmd(nc, in_maps, core_ids=list(range(NC)))
    # OT[b,j,i,d] -> out[b,i,j,d]
    ot = np.concatenate([res.results[c]["OUT"] for c in range(NC)], axis=0)
    out = np.swapaxes(ot.astype(np.float32), 1, 2)
    return np.ascontiguousarray(out, dtype=np.float32)

